# revision 9
# baseline (speedup 1.0000x reference)
"""Trainium2 Bass kernel for ContextQueryAttention (trilinear similarity +
row/col softmax attention).

Full-input contract: kernel(**inputs) takes the complete arrays
  q  [16, 128, 512]   f32
  c  [16, 128, 4096]  f32
  w1 [1, 128] w2 [1, 128] w3 [1, 128] f32
and returns out [16, 512, 4096] f32 = concat([c, a, c*a, c*b], axis=1).

Sharding: data-parallel over batch B=16 across 8 NeuronCores (2 batches per
core), no collectives.

Math notes (used to avoid transposes / extra passes):
  s[n,m] = out3[n,m] + out1[m] + out2[n]
  expS = exp(s) serves BOTH softmaxes: per-row and per-column constants
  cancel in each normalization.
  out3 + out1 = (w3*c + w1)^T @ q          (fold w1 into the lhsT)
  out3^T + out2 = (w3*q + w2)^T @ c        (fold w2 into the lhsT)
  The remaining bias in each layout is per-partition -> free via ACT bias.
  Row sums / col sums come for free from activation accum_out in whichever
  layout has that axis on the free dimension.
"""

import os
import sys

import numpy as np

try:
    import concourse.bass as bass  # noqa: F401
except Exception:  # pragma: no cover
    sys.path.insert(0, "/opt/trn_rl_repo")
    import concourse.bass as bass  # noqa: F401

import concourse.bacc as bacc
import concourse.mybir as mybir
import concourse.tile as tile
from concourse.masks import make_identity

F32 = mybir.dt.float32

# Problem geometry (hardcoded per contract)
B = 16          # total batches
NCORES = 8
CB = B // NCORES  # batches per core = 2
D = 128         # model dim == partition count
M = 512         # query length
N = 4096        # context length
P = 128
NCH = N // P    # 32 n-chunks of 128
MCH = M // P    # 4 m-chunks of 128
NSUB = N // 512  # 8 n-subtiles of 512 (psum free-dim limit)


def build_body(tc, q_ap, c_ap, w1_ap, w2_ap, w3_ap, out_ap):
    """Emit the per-core program. q_ap [CB,128,512], c_ap [CB,128,4096],
    w*_ap [1,128], out_ap [CB,512,4096]."""
    from contextlib import ExitStack

    nc = tc.nc
    mult = mybir.AluOpType.mult
    add = mybir.AluOpType.add
    Exp = mybir.ActivationFunctionType.Exp
    Copy = mybir.ActivationFunctionType.Copy

    with ExitStack() as ctx:
        consts = ctx.enter_context(tc.tile_pool(name="consts", bufs=1))
        cq = ctx.enter_context(tc.tile_pool(name="cq", bufs=2))
        big = ctx.enter_context(tc.tile_pool(name="big", bufs=1))
        sp = ctx.enter_context(tc.tile_pool(name="sp", bufs=1))
        stats = ctx.enter_context(tc.tile_pool(name="stats", bufs=2))
        rowp = ctx.enter_context(tc.tile_pool(name="rowp", bufs=1))
        chunkp = ctx.enter_context(tc.tile_pool(name="chunkp", bufs=3))
        expsp = ctx.enter_context(tc.tile_pool(name="expsp", bufs=3))
        outp = ctx.enter_context(tc.tile_pool(name="outp", bufs=6))
        pp_mm = ctx.enter_context(tc.tile_pool(name="pp_mm", bufs=2, space="PSUM"))
        pp_acc = ctx.enter_context(tc.tile_pool(name="pp_acc", bufs=1, space="PSUM"))
        pp_tr = ctx.enter_context(tc.tile_pool(name="pp_tr", bufs=2, space="PSUM"))

        identity = consts.tile([P, P], F32)
        make_identity(nc, identity)
        w1c = consts.tile([P, 1], F32)
        w2c = consts.tile([P, 1], F32)
        w3c = consts.tile([P, 1], F32)
        nc.sync.dma_start(out=w1c, in_=w1_ap.rearrange("o d -> d o"))
        nc.sync.dma_start(out=w2c, in_=w2_ap.rearrange("o d -> d o"))
        nc.sync.dma_start(out=w3c, in_=w3_ap.rearrange("o d -> d o"))

        for b in range(CB):
            # ---- loads ----
            q_t = cq.tile([P, M], F32, tag="q")
            c_t = cq.tile([P, N], F32, tag="c")
            nc.sync.dma_start(out=q_t, in_=q_ap[b])
            nc.sync.dma_start(out=c_t, in_=c_ap[b])

            # ---- folded lhsT tensor (T layout) ----
            # Bq[d,m] = w3[d]*q[d,m] + w2[d] -> Bq^T @ c = out3^T + out2
            Bq_t = sp.tile([P, M], F32, tag="Bq")
            nc.vector.tensor_scalar(Bq_t, q_t, w3c, w2c, mult, add)

            # ---- per-partition bias columns ----
            # out2col[n] = sum_d w2[d] c[d,n], chunked [128,1] x 32
            ps_o2 = pp_tr.tile([P, NCH], F32, tag="tr")
            for j in range(NCH):
                nc.tensor.matmul(
                    ps_o2[:, j : j + 1],
                    lhsT=c_t[:, j * P : (j + 1) * P],
                    rhs=w2c,
                    start=True,
                    stop=True,
                )
            out2col = stats.tile([P, NCH], F32)
            nc.vector.tensor_copy(out2col, ps_o2)
            # out1col[m] = sum_d w1[d] q[d,m], chunked [128,1] x 4
            ps_o1 = pp_tr.tile([P, MCH], F32, tag="tr")
            for i in range(MCH):
                nc.tensor.matmul(
                    ps_o1[:, i : i + 1],
                    lhsT=q_t[:, i * P : (i + 1) * P],
                    rhs=w1c,
                    start=True,
                    stop=True,
                )
            out1col = stats.tile([P, MCH], F32)
            nc.vector.tensor_copy(out1col, ps_o1)

            # ---- transpose of q (PE transpose via identity) ----
            qT_t = sp.tile([P, M], F32, tag="qT")  # chunk i: qT[mm, i*128+dd]
            for i in range(MCH):
                ps = pp_tr.tile([P, P], F32, tag="tr")
                nc.tensor.transpose(ps, q_t[:, i * P : (i + 1) * P], identity)
                nc.vector.tensor_copy(qT_t[:, i * P : (i + 1) * P], ps)

            # ---- [n,m] layout pass: expS chunks -> tmpU accumulation ----
            rowsumU = stats.tile([P, NCH], F32)
            ps_tmpU = pp_acc.tile([P, M], F32, tag="tmpU")
            for j in range(NCH):
                # A chunk [d, nn]: w3*c + w1 (fold of out1 into the lhsT)
                A_t = chunkp.tile([P, P], F32, tag="A")
                nc.vector.tensor_scalar(
                    A_t, c_t[:, j * P : (j + 1) * P], w3c, w1c, mult, add
                )
                # cT chunk: cT[nn, dd] = c[dd, j*128+nn]
                ps_ct = pp_tr.tile([P, P], F32, tag="tr")
                nc.tensor.transpose(ps_ct, c_t[:, j * P : (j + 1) * P], identity)
                cT_t = chunkp.tile([P, P], F32, tag="cT")
                nc.vector.tensor_copy(cT_t, ps_ct)

                ps_nm = pp_mm.tile([P, M], F32, tag="mm")
                nc.tensor.matmul(ps_nm, lhsT=A_t, rhs=q_t, start=True, stop=True)
                expS_t = expsp.tile([P, M], F32, tag="expS")
                nc.scalar.activation(
                    expS_t,
                    ps_nm,
                    Exp,
                    bias=out2col[:, j : j + 1],
                    scale=1.0,
                    accum_out=rowsumU[:, j : j + 1],
                )
                nc.tensor.matmul(
                    ps_tmpU,
                    lhsT=cT_t,
                    rhs=expS_t,
                    start=(j == 0),
                    stop=(j == NCH - 1),
                )

            # ---- [m,n] layout pass: expST (resident) + colsum ----
            expST_t = big.tile([P, MCH, N], F32, tag="expST")
            colsumU8 = stats.tile([P, MCH, NSUB], F32)
            for i in range(MCH):
                for js in range(NSUB):
                    ps_T = pp_mm.tile([P, 512], F32, tag="mm")
                    nc.tensor.matmul(
                        ps_T,
                        lhsT=Bq_t[:, i * P : (i + 1) * P],
                        rhs=c_t[:, js * 512 : (js + 1) * 512],
                        start=True,
                        stop=True,
                    )
                    nc.scalar.activation(
                        expST_t[:, i, js * 512 : (js + 1) * 512],
                        ps_T,
                        Exp,
                        bias=out1col[:, i : i + 1],
                        scale=1.0,
                        accum_out=colsumU8[:, i, js : js + 1],
                    )

            # ---- softmax stats ----
            colsum = stats.tile([P, MCH], F32)
            nc.vector.reduce_sum(colsum, colsumU8, axis=mybir.AxisListType.X)
            colinv = stats.tile([P, MCH], F32)
            nc.vector.reciprocal(colinv, colsum)
            rowinv = stats.tile([P, NCH], F32)
            nc.vector.reciprocal(rowinv, rowsumU)

            # ---- tmp^T with column-softmax normalization folded in ----
            tmpU_t = sp.tile([P, M], F32, tag="tmpU_s")
            nc.vector.tensor_copy(tmpU_t, ps_tmpU)
            tmpT_t = sp.tile([P, M], F32, tag="tmpT")
            for i in range(MCH):
                ps = pp_tr.tile([P, P], F32, tag="tr")
                nc.tensor.transpose(ps, tmpU_t[:, i * P : (i + 1) * P], identity)
                nc.scalar.activation(
                    tmpT_t[:, i * P : (i + 1) * P],
                    ps,
                    Copy,
                    scale=colinv[:, i : i + 1],
                )

            # ---- broadcast 1/rowsum along partitions: [128, N] ----
            ps_r = pp_tr.tile([NCH, P], F32, tag="tr")
            nc.tensor.transpose(ps_r, rowinv, identity)  # [32,128]: part j, col nn
            rowT = stats.tile([NCH, P], F32)
            nc.vector.tensor_copy(rowT, ps_r)
            rowrow = rowp.tile([1, N], F32, tag="rowrow")
            # [32,128] -> [1,4096] with col = j*128+nn  (32 contiguous 512B runs)
            nc.sync.dma_start(
                out=rowrow.rearrange("p (a b) -> p a b", a=NCH), in_=rowT
            )
            rowinvb = big.tile([P, N], F32, tag="rowinvb")
            nc.gpsimd.partition_broadcast(rowinvb, rowrow)

            # crow[d,n] = c[d,n] / rowsum[n]  (gpsimd: keeps DVE free)
            crow = big.tile([P, N], F32, tag="crow")
            nc.gpsimd.tensor_tensor(crow, c_t, rowinvb, mult)

            # ---- outputs ----
            for js in range(NSUB):
                lo, hi = js * 512, (js + 1) * 512
                # a = (expST^T-contraction with q) / rowsum
                ps_a = pp_mm.tile([P, 512], F32, tag="ab")
                for i in range(MCH):
                    nc.tensor.matmul(
                        ps_a,
                        lhsT=qT_t[:, i * P : (i + 1) * P],
                        rhs=expST_t[:, i, lo:hi],
                        start=(i == 0),
                        stop=(i == MCH - 1),
                    )
                a_t = outp.tile([P, 512], F32, tag="outt")
                nc.vector.tensor_tensor(a_t, ps_a, rowinvb[:, lo:hi], mult)
                nc.sync.dma_start(out=out_ap[b, P : 2 * P, lo:hi], in_=a_t)
                ca_t = outp.tile([P, 512], F32, tag="outt")
                nc.vector.tensor_tensor(ca_t, ps_a, crow[:, lo:hi], mult)
                nc.sync.dma_start(out=out_ap[b, 2 * P : 3 * P, lo:hi], in_=ca_t)

                ps_b = pp_mm.tile([P, 512], F32, tag="ab")
                for i in range(MCH):
                    nc.tensor.matmul(
                        ps_b,
                        lhsT=tmpT_t[:, i * P : (i + 1) * P],
                        rhs=expST_t[:, i, lo:hi],
                        start=(i == 0),
                        stop=(i == MCH - 1),
                    )
                cb_t = outp.tile([P, 512], F32, tag="outt")
                nc.vector.tensor_tensor(cb_t, ps_b, crow[:, lo:hi], mult)
                nc.sync.dma_start(out=out_ap[b, 3 * P : 4 * P, lo:hi], in_=cb_t)

                # block 0 is just c
                nc.sync.dma_start(out=out_ap[b, 0:P, lo:hi], in_=c_t[:, lo:hi])


_PROGRAM = None


def _get_program():
    global _PROGRAM
    if _PROGRAM is not None:
        return _PROGRAM
    nc = bacc.Bacc("TRN2", target_bir_lowering=False, debug=False, num_devices=NCORES)
    q_d = nc.dram_tensor("q", [CB, D, M], F32, kind="ExternalInput")
    c_d = nc.dram_tensor("c", [CB, D, N], F32, kind="ExternalInput")
    w1_d = nc.dram_tensor("w1", [1, D], F32, kind="ExternalInput")
    w2_d = nc.dram_tensor("w2", [1, D], F32, kind="ExternalInput")
    w3_d = nc.dram_tensor("w3", [1, D], F32, kind="ExternalInput")
    out_d = nc.dram_tensor("out", [CB, 4 * D, N], F32, kind="ExternalOutput")
    with tile.TileContext(nc) as tc:
        build_body(
            tc, q_d.ap(), c_d.ap(), w1_d.ap(), w2_d.ap(), w3_d.ap(), out_d.ap()
        )
    nc.compile()
    _PROGRAM = nc
    return nc


def kernel(q, c, w1, w2, w3, _collect_results=None):
    q = np.ascontiguousarray(q, dtype=np.float32)
    c = np.ascontiguousarray(c, dtype=np.float32)
    w1 = np.ascontiguousarray(w1, dtype=np.float32)
    w2 = np.ascontiguousarray(w2, dtype=np.float32)
    w3 = np.ascontiguousarray(w3, dtype=np.float32)

    nc = _get_program()
    in_maps = [
        {
            "q": q[CB * i : CB * (i + 1)],
            "c": c[CB * i : CB * (i + 1)],
            "w1": w1,
            "w2": w2,
            "w3": w3,
        }
        for i in range(NCORES)
    ]
    from concourse import bass_utils

    res = bass_utils.run_bass_kernel_spmd(nc, in_maps, core_ids=list(range(NCORES)))
    if _collect_results is not None:
        _collect_results.append(res)
    return np.concatenate([r["out"] for r in res.results], axis=0)


# revision 10
# speedup vs baseline: 6552.0713x; 6552.0713x over previous
"""Trainium2 Bass kernel for ContextQueryAttention (trilinear similarity +
row/col softmax attention).

Full-input contract: kernel(**inputs) takes the complete arrays
  q  [16, 128, 512]   f32
  c  [16, 128, 4096]  f32
  w1 [1, 128] w2 [1, 128] w3 [1, 128] f32
and returns out [16, 512, 4096] f32 = concat([c, a, c*a, c*b], axis=1).

Sharding: data-parallel over batch B=16 across 8 NeuronCores (2 batches per
core), no collectives.

Math notes (used to avoid transposes / extra passes):
  s[n,m] = out3[n,m] + out1[m] + out2[n]
  expS = exp(s) serves BOTH softmaxes: per-row and per-column constants
  cancel in each normalization.
  out3 + out1 = (w3*c + w1)^T @ q          (fold w1 into the lhsT)
  out3^T + out2 = (w3*q + w2)^T @ c        (fold w2 into the lhsT)
  The remaining bias in each layout is per-partition -> free via ACT bias.
  Row sums / col sums come for free from activation accum_out in whichever
  layout has that axis on the free dimension.
"""

import os
import sys

import numpy as np

try:
    import concourse.bass as bass  # noqa: F401
except Exception:  # pragma: no cover
    sys.path.insert(0, "/opt/trn_rl_repo")
    import concourse.bass as bass  # noqa: F401

import concourse.bacc as bacc
import concourse.mybir as mybir
import concourse.tile as tile
from concourse.masks import make_identity

F32 = mybir.dt.float32

# Problem geometry (hardcoded per contract)
B = 16          # total batches
NCORES = 8
CB = B // NCORES  # batches per core = 2
D = 128         # model dim == partition count
M = 512         # query length
N = 4096        # context length
P = 128
NCH = N // P    # 32 n-chunks of 128
MCH = M // P    # 4 m-chunks of 128
NSUB = N // 512  # 8 n-subtiles of 512 (psum free-dim limit)


def build_body(tc, q_ap, c_ap, w1_ap, w2_ap, w3_ap, out_ap):
    """Emit the per-core program. q_ap [CB,128,512], c_ap [CB,128,4096],
    w*_ap [1,128], out_ap [CB,512,4096]."""
    from contextlib import ExitStack

    nc = tc.nc
    mult = mybir.AluOpType.mult
    add = mybir.AluOpType.add
    Exp = mybir.ActivationFunctionType.Exp
    Copy = mybir.ActivationFunctionType.Copy

    with ExitStack() as ctx:
        consts = ctx.enter_context(tc.tile_pool(name="consts", bufs=1))
        cq = ctx.enter_context(tc.tile_pool(name="cq", bufs=2))
        big = ctx.enter_context(tc.tile_pool(name="big", bufs=1))
        sp = ctx.enter_context(tc.tile_pool(name="sp", bufs=1))
        stats = ctx.enter_context(tc.tile_pool(name="stats", bufs=2))
        rowp = ctx.enter_context(tc.tile_pool(name="rowp", bufs=1))
        chunkp = ctx.enter_context(tc.tile_pool(name="chunkp", bufs=3))
        expsp = ctx.enter_context(tc.tile_pool(name="expsp", bufs=3))
        outp = ctx.enter_context(tc.tile_pool(name="outp", bufs=6))
        pp_mm = ctx.enter_context(tc.tile_pool(name="pp_mm", bufs=2, space="PSUM"))
        pp_acc = ctx.enter_context(tc.tile_pool(name="pp_acc", bufs=1, space="PSUM"))
        pp_tr = ctx.enter_context(tc.tile_pool(name="pp_tr", bufs=2, space="PSUM"))

        identity = consts.tile([P, P], F32)
        make_identity(nc, identity)
        w1c = consts.tile([P, 1], F32)
        w2c = consts.tile([P, 1], F32)
        w3c = consts.tile([P, 1], F32)
        nc.sync.dma_start(out=w1c, in_=w1_ap.rearrange("o d -> d o"))
        nc.sync.dma_start(out=w2c, in_=w2_ap.rearrange("o d -> d o"))
        nc.sync.dma_start(out=w3c, in_=w3_ap.rearrange("o d -> d o"))

        for b in range(CB):
            # ---- loads ----
            q_t = cq.tile([P, M], F32, tag="q")
            c_t = cq.tile([P, N], F32, tag="c")
            nc.sync.dma_start(out=q_t, in_=q_ap[b])
            nc.sync.dma_start(out=c_t, in_=c_ap[b])

            # ---- folded lhsT tensor (T layout) ----
            # Bq[d,m] = w3[d]*q[d,m] + w2[d] -> Bq^T @ c = out3^T + out2
            Bq_t = sp.tile([P, M], F32, tag="Bq")
            nc.vector.tensor_scalar(Bq_t, q_t, w3c, w2c, mult, add)

            # ---- per-partition bias columns ----
            # out2col[n] = sum_d w2[d] c[d,n], chunked [128,1] x 32
            ps_o2 = pp_tr.tile([P, NCH], F32, tag="tr")
            for j in range(NCH):
                nc.tensor.matmul(
                    ps_o2[:, j : j + 1],
                    lhsT=c_t[:, j * P : (j + 1) * P],
                    rhs=w2c,
                    start=True,
                    stop=True,
                )
            out2col = stats.tile([P, NCH], F32)
            nc.vector.tensor_copy(out2col, ps_o2)
            # out1col[m] = sum_d w1[d] q[d,m], chunked [128,1] x 4
            ps_o1 = pp_tr.tile([P, MCH], F32, tag="tr")
            for i in range(MCH):
                nc.tensor.matmul(
                    ps_o1[:, i : i + 1],
                    lhsT=q_t[:, i * P : (i + 1) * P],
                    rhs=w1c,
                    start=True,
                    stop=True,
                )
            out1col = stats.tile([P, MCH], F32)
            nc.vector.tensor_copy(out1col, ps_o1)

            # ---- transpose of q (PE transpose via identity) ----
            qT_t = sp.tile([P, M], F32, tag="qT")  # chunk i: qT[mm, i*128+dd]
            for i in range(MCH):
                ps = pp_tr.tile([P, P], F32, tag="tr")
                nc.tensor.transpose(ps, q_t[:, i * P : (i + 1) * P], identity)
                nc.vector.tensor_copy(qT_t[:, i * P : (i + 1) * P], ps)

            # ---- [n,m] layout pass: expS chunks -> tmpU accumulation ----
            rowsumU = stats.tile([P, NCH], F32)
            ps_tmpU = pp_acc.tile([P, M], F32, tag="tmpU")
            for j in range(NCH):
                # A chunk [d, nn]: w3*c + w1 (fold of out1 into the lhsT)
                A_t = chunkp.tile([P, P], F32, tag="A")
                nc.vector.tensor_scalar(
                    A_t, c_t[:, j * P : (j + 1) * P], w3c, w1c, mult, add
                )
                # cT chunk: cT[nn, dd] = c[dd, j*128+nn]
                ps_ct = pp_tr.tile([P, P], F32, tag="tr")
                nc.tensor.transpose(ps_ct, c_t[:, j * P : (j + 1) * P], identity)
                cT_t = chunkp.tile([P, P], F32, tag="cT")
                nc.vector.tensor_copy(cT_t, ps_ct)

                ps_nm = pp_mm.tile([P, M], F32, tag="mm")
                nc.tensor.matmul(ps_nm, lhsT=A_t, rhs=q_t, start=True, stop=True)
                expS_t = expsp.tile([P, M], F32, tag="expS")
                nc.scalar.activation(
                    expS_t,
                    ps_nm,
                    Exp,
                    bias=out2col[:, j : j + 1],
                    scale=1.0,
                    accum_out=rowsumU[:, j : j + 1],
                )
                nc.tensor.matmul(
                    ps_tmpU,
                    lhsT=cT_t,
                    rhs=expS_t,
                    start=(j == 0),
                    stop=(j == NCH - 1),
                )

            # ---- [m,n] layout pass: expST (resident) + colsum ----
            expST_t = big.tile([P, MCH, N], F32, tag="expST")
            colsumU8 = stats.tile([P, MCH, NSUB], F32)
            for i in range(MCH):
                for js in range(NSUB):
                    ps_T = pp_mm.tile([P, 512], F32, tag="mm")
                    nc.tensor.matmul(
                        ps_T,
                        lhsT=Bq_t[:, i * P : (i + 1) * P],
                        rhs=c_t[:, js * 512 : (js + 1) * 512],
                        start=True,
                        stop=True,
                    )
                    nc.scalar.activation(
                        expST_t[:, i, js * 512 : (js + 1) * 512],
                        ps_T,
                        Exp,
                        bias=out1col[:, i : i + 1],
                        scale=1.0,
                        accum_out=colsumU8[:, i, js : js + 1],
                    )

            # ---- softmax stats ----
            colsum = stats.tile([P, MCH], F32)
            nc.vector.reduce_sum(colsum, colsumU8, axis=mybir.AxisListType.X)
            colinv = stats.tile([P, MCH], F32)
            nc.vector.reciprocal(colinv, colsum)
            rowinv = stats.tile([P, NCH], F32)
            nc.vector.reciprocal(rowinv, rowsumU)

            # ---- tmp^T with column-softmax normalization folded in ----
            tmpU_t = sp.tile([P, M], F32, tag="tmpU_s")
            nc.vector.tensor_copy(tmpU_t, ps_tmpU)
            tmpT_t = sp.tile([P, M], F32, tag="tmpT")
            for i in range(MCH):
                ps = pp_tr.tile([P, P], F32, tag="tr")
                nc.tensor.transpose(ps, tmpU_t[:, i * P : (i + 1) * P], identity)
                nc.scalar.activation(
                    tmpT_t[:, i * P : (i + 1) * P],
                    ps,
                    Copy,
                    scale=colinv[:, i : i + 1],
                )

            # ---- broadcast 1/rowsum along partitions: [128, N] ----
            ps_r = pp_tr.tile([NCH, P], F32, tag="tr")
            nc.tensor.transpose(ps_r, rowinv, identity)  # [32,128]: part j, col nn
            rowT = stats.tile([NCH, P], F32)
            nc.vector.tensor_copy(rowT, ps_r)
            rowrow = rowp.tile([1, N], F32, tag="rowrow")
            # [32,128] -> [1,4096] with col = j*128+nn  (32 contiguous 512B runs)
            nc.sync.dma_start(
                out=rowrow.rearrange("p (a b) -> p a b", a=NCH), in_=rowT
            )
            rowinvb = big.tile([P, N], F32, tag="rowinvb")
            nc.gpsimd.partition_broadcast(rowinvb, rowrow)

            # crow[d,n] = c[d,n] / rowsum[n]  (gpsimd: keeps DVE free)
            crow = big.tile([P, N], F32, tag="crow")
            nc.gpsimd.tensor_tensor(crow, c_t, rowinvb, mult)

            # ---- outputs ----
            for js in range(NSUB):
                lo, hi = js * 512, (js + 1) * 512
                # a = (expST^T-contraction with q) / rowsum
                ps_a = pp_mm.tile([P, 512], F32, tag="ab")
                for i in range(MCH):
                    nc.tensor.matmul(
                        ps_a,
                        lhsT=qT_t[:, i * P : (i + 1) * P],
                        rhs=expST_t[:, i, lo:hi],
                        start=(i == 0),
                        stop=(i == MCH - 1),
                    )
                a_t = outp.tile([P, 512], F32, tag="outt")
                nc.vector.tensor_tensor(a_t, ps_a, rowinvb[:, lo:hi], mult)
                nc.sync.dma_start(out=out_ap[b, P : 2 * P, lo:hi], in_=a_t)
                ca_t = outp.tile([P, 512], F32, tag="outt")
                nc.vector.tensor_tensor(ca_t, ps_a, crow[:, lo:hi], mult)
                nc.sync.dma_start(out=out_ap[b, 2 * P : 3 * P, lo:hi], in_=ca_t)

                ps_b = pp_mm.tile([P, 512], F32, tag="ab")
                for i in range(MCH):
                    nc.tensor.matmul(
                        ps_b,
                        lhsT=tmpT_t[:, i * P : (i + 1) * P],
                        rhs=expST_t[:, i, lo:hi],
                        start=(i == 0),
                        stop=(i == MCH - 1),
                    )
                cb_t = outp.tile([P, 512], F32, tag="outt")
                nc.vector.tensor_tensor(cb_t, ps_b, crow[:, lo:hi], mult)
                nc.sync.dma_start(out=out_ap[b, 3 * P : 4 * P, lo:hi], in_=cb_t)

                # block 0 is just c
                nc.sync.dma_start(out=out_ap[b, 0:P, lo:hi], in_=c_t[:, lo:hi])


_PROGRAM = None


def _build_program(loops=None):
    """Build the per-core Bass program. loops=None -> straight-line (grading
    path); loops=R -> wrap the body in a Tile For_i repetition loop (used
    only for steady-state benchmarking)."""
    nc = bacc.Bacc("TRN2", target_bir_lowering=False, debug=False, num_devices=NCORES)
    q_d = nc.dram_tensor("q", [CB, D, M], F32, kind="ExternalInput")
    c_d = nc.dram_tensor("c", [CB, D, N], F32, kind="ExternalInput")
    w1_d = nc.dram_tensor("w1", [1, D], F32, kind="ExternalInput")
    w2_d = nc.dram_tensor("w2", [1, D], F32, kind="ExternalInput")
    w3_d = nc.dram_tensor("w3", [1, D], F32, kind="ExternalInput")
    out_d = nc.dram_tensor("out", [CB, 4 * D, N], F32, kind="ExternalOutput")
    with tile.TileContext(nc) as tc:
        if loops is None:
            build_body(
                tc, q_d.ap(), c_d.ap(), w1_d.ap(), w2_d.ap(), w3_d.ap(), out_d.ap()
            )
        else:
            with tc.For_i(0, loops, 1):
                build_body(
                    tc,
                    q_d.ap(),
                    c_d.ap(),
                    w1_d.ap(),
                    w2_d.ap(),
                    w3_d.ap(),
                    out_d.ap(),
                )
    nc.compile()
    return nc


def _get_program():
    global _PROGRAM
    if _PROGRAM is None:
        _PROGRAM = _build_program()
    return _PROGRAM


def kernel(q, c, w1, w2, w3, _collect_results=None):
    q = np.ascontiguousarray(q, dtype=np.float32)
    c = np.ascontiguousarray(c, dtype=np.float32)
    w1 = np.ascontiguousarray(w1, dtype=np.float32)
    w2 = np.ascontiguousarray(w2, dtype=np.float32)
    w3 = np.ascontiguousarray(w3, dtype=np.float32)

    nc = _get_program()
    in_maps = [
        {
            "q": q[CB * i : CB * (i + 1)],
            "c": c[CB * i : CB * (i + 1)],
            "w1": w1,
            "w2": w2,
            "w3": w3,
        }
        for i in range(NCORES)
    ]
    from concourse import bass_utils

    res = bass_utils.run_bass_kernel_spmd(nc, in_maps, core_ids=list(range(NCORES)))
    if _collect_results is not None:
        _collect_results.append(res)
    return np.concatenate([r["out"] for r in res.results], axis=0)
